# revision 34
# baseline (speedup 1.0000x reference)
"""ConvolutionalFilterManifold Trainium2 kernel.

Reference: a tiny "manifold" MLP maps q[B,1,8,8] -> per-sample 3x3 conv
filters w[B,8,8,3,3] and biases b[B,8]; the heavy op is a per-sample
conv2d over x[B,8,512,512] (pad 1, stride 1) -> y[B,8,512,512].

Strategy: manifold on host (tiny, exact); conv on 8 NeuronCores with
pure batch data-parallelism (4 samples/core). Default MODE "s4" is
pure bf16 (rel err ~2.9e-3 against the 2e-2 gate):

- Per output row-group G of TOUT=14 rows, stationary A_dx[(ri*8+ic),
  (ro*8+oc)] = w[oc, ic, ri-ro, dx] (banded block-Toeplitz, K=128 =
  16 input rows x 8 in-chans, M=112 = 14 out rows x 8 out-chans);
  3 dx taps = 3 PSUM-accumulating bf16 matmuls per (group, sample).
  Center tap runs full N=512 with start=True; dx=0/2 use clipped
  psum/rhs ranges (zero width-padding semantics). Bias rides the
  PSUM->SBUF copy (DVE tensor_scalar_add / Act Identity+bias).
- DMA layout is the key: x is host-prepped to [HP=520, IC, SPC, W]
  bf16 with the 4 samples interleaved inside each (row, ic) line, so
  every SBUF partition line is exactly 4096 B (the DMA packet sweet
  spot: ~23 GB/s/engine x 16 engines) and each 16-row group slab is
  one fully contiguous 512 KB read, split into two 64-line halves on
  separate queue rings (sync/gpsimd). Output goes out as bf16 in SBUF
  tile layout [NGRP, 112, SPC*W] (4 KB lines, scalar-ring DMA) and is
  unscrambled + upcast on host. Row zero-padding is baked into the
  DRAM image (rows 0 and 513+), so all 37 slabs are uniform: no edge
  variants, no memsets, no bias matmul.

Hardware constraint discovered empirically: every TPB instruction has
ONE sync-wait slot (bf16 matmuls get 2 via the LDW+MM split; 4-byte
self-loading matmuls get just 1). The emission order below keeps every
instruction's Tile-assigned wait count within its slots, and the
TileContext drain is patched to spread its per-proc waits over nops.
"""

import os
import re
import sys

sys.path.insert(0, "/opt/trn_rl_repo")

import numpy as np  # noqa: E402

import bass_rust  # noqa: E402
import concourse.bass as bass  # noqa: E402
import concourse.mybir as mybir  # noqa: E402
from concourse.bass_utils import run_bass_kernel_spmd  # noqa: E402
from concourse.tile import TileContext  # noqa: E402
from concourse.vector_clock import ScopedClock  # noqa: E402

B, IC, OC = 32, 8, 8
H = W = 512
NCORES = 8
SPC = B // NCORES  # samples per core
TOUT = 14  # output rows per group
TIN = 16  # input rows per group (TOUT + 2)
NGRP = 37  # 36 full groups + one 8-row group
M_PART = OC * TOUT  # 112 psum partitions
# (group-start, n-groups) chunks; 4 groups -> 4 PSUM banks, x2 bufs = 8
CHUNKS = [(g, min(4, NGRP - g)) for g in range(0, NGRP, 4)]

MODE = os.environ.get("CFM_MODE", "s4")  # s4 | bf16 | split | f32r | f32 | pair

_ORIG_DRAIN = TileContext._drain_and_barrier


def _patched_drain_and_barrier(self, tick_clock, wait_clock):
    gc = tick_clock.global_clock
    vals = [int(v) for v in re.findall(r"-?\d+", repr(gc))]
    for i, v in enumerate(vals):
        if v > 0:
            sub = [0] * len(vals)
            sub[i] = v
            nop = self.nc.sync.nop(nofuse=True)
            wait_clock.add_sem_waits(
                nop.ins, ScopedClock({None: bass_rust.VectorClock(sub)})
            )
    self.nc.sync.drain()
    self.nc.all_engine_barrier()
    assert self.sems is not None
    popped = self.nc._tile_sem_poison_stack.pop()
    assert popped is self._sem_poison
    self.nc.clear_and_free_semaphores(list(self.sems.allocated().values()))
    self.nc.all_engine_barrier()


TileContext._drain_and_barrier = _patched_drain_and_barrier


def _legalize_waits(nc):
    """Every TPB instruction encodes at most ONE sync wait. Tile can
    attach several (multi-queue DMA producers, tile-granular WAR
    fan-ins). Hoist the excess onto same-engine InstNoOps inserted
    right before the instruction — the engine then blocks on the same
    sem set, just sequentially."""
    for fn in nc.m.functions:
        for bb in fn.blocks:
            out, changed = [], False
            for inst in bb.instructions:
                si = inst.sync_info
                if si is not None and len(si.on_wait) > 1:
                    waits = list(si.on_wait)
                    for w in waits[:-1]:
                        out.append(
                            mybir.InstNoOp(
                                name=nc.get_next_instruction_name(),
                                engine=inst.engine,
                                bass_nofuse=True,
                                sync_info=mybir.SyncInfo(on_wait=[w], on_update=[]),
                            )
                        )
                    inst.sync_info = mybir.SyncInfo(
                        on_wait=waits[-1:], on_update=list(si.on_update)
                    )
                    changed = True
                out.append(inst)
            if changed:
                bb.instructions = out


def _dram_ap(t, ap_list, offset):
    a = t[:].copy()
    a.ap = bass_rust.VecI64Pair(ap_list)
    a.offset = offset
    return a


def _manifold(q, wm1, bm1, wm2, bm2, wt, bt, wb, bb):
    m = np.einsum("bihw,cihw->bc", q, wm1) + bm1
    m = np.where(m > 0, m, np.float32(0.01) * m).astype(np.float32)
    m = m @ wm2.T + bm2
    m = np.where(m > 0, m, np.float32(0.01) * m).astype(np.float32)
    w = np.einsum("bc,cokl->bokl", m, wt) + bt[None, :, None, None]
    w = w.reshape(B, OC, IC, 3, 3).astype(np.float32)
    b = (m @ wb.T + bb).astype(np.float32)
    return w, b


def _build_stationaries(w):
    """w: [B, OC, IC, 3, 3] -> A: [B, 3 variants, 3 dx, 128, 112] f32.

    A[s, v, dx, ri*8+ic, ro*8+oc] = w[s, oc, ic, ri-ro, dx] for
    0 <= ri-ro <= 2 else 0. Variant 1 zeroes rows ri=0 (G=0, row -1);
    variant 2 zeroes rows ri>=9 (G=36, rows >=512)."""
    A = np.zeros((B, 3, 3, 128, M_PART), np.float32)
    ro = np.arange(TOUT)
    for dy in range(3):
        ri = ro + dy  # 14 values in [dy, dy+13]
        # block [ri*8 + ic, ro*8 + oc] = w[:, oc, ic, dy, dx]
        # use advanced indexing over (ro, ic, oc)
        blk = w[:, :, :, dy, :]  # [B, OC, IC, 3dx]
        for t in range(TOUT):
            A[:, 0, :, (t + dy) * 8 : (t + dy) * 8 + 8, t * 8 : t * 8 + 8] = (
                blk.transpose(0, 3, 2, 1)  # [B, dx, IC, OC]
            )
    A[:, 1] = A[:, 0]
    A[:, 1, :, 0:8, :] = 0.0
    A[:, 2] = A[:, 0]
    A[:, 2, :, 72:, :] = 0.0
    return A


def _build_stationaries_pair(w):
    """Row-pair variant: A[B, 3v, 3dx, 2parity, 64, 112] with
    A[s,v,dx,p, rq*8+ic, ro*8+oc] = w[s,oc,ic, 2rq+p-ro, dx] for
    0 <= 2rq+p-ro <= 2. Variant 1 zeroes input row index ri=2rq+p == 0;
    variant 2 zeroes ri >= 9."""
    A = np.zeros((B, 3, 3, 2, 64, M_PART), np.float32)
    for p in range(2):
        for rq in range(8):
            ri = 2 * rq + p
            for ro in range(TOUT):
                dy = ri - ro
                if 0 <= dy <= 2:
                    A[:, 0, :, p, rq * 8 : rq * 8 + 8, ro * 8 : ro * 8 + 8] = w[
                        :, :, :, dy, :
                    ].transpose(0, 3, 2, 1)
    A[:, 1] = A[:, 0]
    A[:, 1, :, 0, 0:8, :] = 0.0  # ri = 0
    A[:, 2] = A[:, 0]
    for p in range(2):
        for rq in range(8):
            if 2 * rq + p >= 9:
                A[:, 2, :, p, rq * 8 : rq * 8 + 8, :] = 0.0
    return A


if MODE == "pair":
    _A_COLS = SPC * 3 * 3 * 2 * M_PART  # 8064
else:
    _A_COLS = SPC * 3 * 3 * M_PART  # 4032
_CA_COLS = SPC * 3 * M_PART  # 1344 (bf16 mode: no edge variants)
_ONES_OFF = _A_COLS
_BIAS_OFF = _A_COLS + W
_BVEC_OFF = _A_COLS  # f32r mode: per-partition bias vectors instead of ones/row
_CONST_COLS = _A_COLS + SPC if MODE == "f32r" else _BIAS_OFF + SPC * M_PART


def _build_consts(A_core, b_core):
    """Pack per-core consts into one [128, _CONST_COLS] f32 image."""
    C = np.zeros((128, _CONST_COLS), np.float32)
    if MODE == "pair":
        C[:64, :_A_COLS] = A_core.transpose(4, 0, 1, 2, 3, 5).reshape(64, _A_COLS)
    else:
        C[:, :_A_COLS] = A_core.transpose(3, 0, 1, 2, 4).reshape(128, _A_COLS)
    if MODE == "f32r":
        # bias as [112, SPC] per-partition column vectors (DVE adds them)
        for s in range(SPC):
            C[0:M_PART, _BVEC_OFF + s] = np.tile(b_core[s], TOUT)
    else:
        C[0, _ONES_OFF : _ONES_OFF + W] = 1.0
        bias_block = np.repeat(b_core[:, None, :], TOUT, axis=1)  # [SPC, 14, 8]
        C[0, _BIAS_OFF :] = bias_block.reshape(-1)
    return C


def _a_col(s, v, dx):
    return ((s * 3 + v) * 3 + dx) * M_PART


def _a_col_pair(s, v, dx, p):
    return ((((s * 3 + v) * 3 + dx) * 2) + p) * M_PART


_HW = H * W  # per-channel plane, elements
_SAMP = IC * _HW  # per-sample elements


def _emit_conv(nc, tc, xin, yout, c32, cbf):
    """Emit the per-core conv program.

    xin: DRAM [SPC, IC, H, W]; yout: DRAM [SPC, OC, H, W]
    c32: DRAM [128, _CONST_COLS] (f32r or f32 depending on mode)
    cbf: DRAM [128, 2*_A_COLS] bf16 (split mode only: A_hi | A_lo)
    """
    f32 = mybir.dt.float32
    bf16 = mybir.dt.bfloat16
    cdt = mybir.dt.float32r if MODE in ("split", "f32r") else f32
    xdt = cdt if MODE in ("f32r", "f32") else f32

    import contextlib

    with contextlib.ExitStack() as ctx:
        consts = ctx.enter_context(tc.tile_pool(name="consts", bufs=1))
        inp = ctx.enter_context(tc.tile_pool(name="inp", bufs=3))
        hlp = ctx.enter_context(tc.tile_pool(name="hlp", bufs=3))
        outp = ctx.enter_context(tc.tile_pool(name="outp", bufs=3))
        psum = ctx.enter_context(tc.tile_pool(name="psum", bufs=2, space="PSUM"))

        c32_sb = consts.tile([128, _CONST_COLS], cdt)
        nc.sync.dma_start(out=c32_sb[:], in_=c32[:])
        if MODE == "split":
            cbf_sb = consts.tile([128, 2 * _A_COLS], bf16)
            nc.sync.dma_start(out=cbf_sb[:], in_=cbf[:])

        ones_ap = c32_sb[0:1, _ONES_OFF : _ONES_OFF + W]

        for s in range(SPC):
            for G0, NG in CHUNKS:
                ti = inp.tile([128, 4, W], xdt, tag="ti")
                # ---- input DMAs, one per group (DMA APs max 3 dims)
                for g in range(NG):
                    G = G0 + g
                    if G == 0:
                        # zero the row(-1) slab: stale SBUF could hold NaNs
                        # and 0-weight x NaN still poisons the accumulation
                        nc.vector.memset(ti[0:32, 0:1, :], 0.0)
                        nc.sync.dma_start(
                            out=ti[8:128, 0:1, :],
                            in_=_dram_ap(xin, [[W, 15], [_HW, IC], [1, W]], s * _SAMP),
                        )
                    elif G == NGRP - 1:
                        nc.vector.memset(ti[64:128, g : g + 1, :], 0.0)
                        nc.sync.dma_start(
                            out=ti[0:72, g : g + 1, :],
                            in_=_dram_ap(
                                xin, [[W, 9], [_HW, IC], [1, W]], s * _SAMP + 503 * W
                            ),
                        )
                    else:
                        nc.sync.dma_start(
                            out=ti[:, g : g + 1, :],
                            in_=_dram_ap(
                                xin,
                                [[W, TIN], [_HW, IC], [1, W]],
                                s * _SAMP + (14 * G - 1) * W,
                            ),
                        )
                if MODE == "split":
                    th = hlp.tile([128, 4, W], bf16, tag="th")
                    tl = hlp.tile([128, 4, W], bf16, tag="tl")
                    for g in range(NG):
                        nc.vector.tensor_copy(
                            out=th[:, g : g + 1, :], in_=ti[:, g : g + 1, :]
                        )
                        nc.vector.tensor_sub(
                            out=tl[:, g : g + 1, :],
                            in0=ti[:, g : g + 1, :],
                            in1=th[:, g : g + 1, :],
                        )

                to = outp.tile([M_PART, 4 * W], f32, tag="to")
                ps = psum.tile([M_PART, 4 * W], f32)

                for g in range(NG):
                    G = G0 + g
                    v = 1 if G == 0 else (2 if G == NGRP - 1 else 0)
                    pcol = g * W
                    # bias pre-load: psum[:, :] = bias x ones  (start)
                    nc.tensor.matmul(
                        ps[:, pcol : pcol + W],
                        c32_sb[0:1, _BIAS_OFF + s * M_PART : _BIAS_OFF + (s + 1) * M_PART],
                        ones_ap,
                        start=True,
                        stop=False,
                        skip_group_check=True,
                    )
                    # dx taps: out cols [lo,hi) <- x cols [lo+dx-1, hi+dx-1)
                    taps = []
                    for dx in range(3):
                        lo = max(0, 1 - dx)
                        hi = W - max(0, dx - 1)
                        taps.append((dx, lo, hi))
                    if MODE == "split":
                        mm_ops = []
                        for dx, lo, hi in taps:
                            ah = cbf_sb[:, _a_col(s, v, dx) : _a_col(s, v, dx) + M_PART]
                            al = cbf_sb[
                                :,
                                _A_COLS + _a_col(s, v, dx) : _A_COLS
                                + _a_col(s, v, dx)
                                + M_PART,
                            ]
                            xh = th[:, g, lo + dx - 1 : hi + dx - 1]
                            xl = tl[:, g, lo + dx - 1 : hi + dx - 1]
                            mm_ops.append((ah, xh))
                            mm_ops.append((ah, xl))
                            mm_ops.append((al, xh))
                        for i, (a_ap, x_ap) in enumerate(mm_ops):
                            dx, lo, hi = taps[i // 3]
                            nc.tensor.matmul(
                                ps[:, pcol + lo : pcol + hi],
                                a_ap,
                                x_ap,
                                start=False,
                                stop=(i == len(mm_ops) - 1),
                                skip_group_check=True,
                            )
                    else:
                        for i, (dx, lo, hi) in enumerate(taps):
                            nc.tensor.matmul(
                                ps[:, pcol + lo : pcol + hi],
                                c32_sb[:, _a_col(s, v, dx) : _a_col(s, v, dx) + M_PART],
                                ti[:, g, lo + dx - 1 : hi + dx - 1],
                                start=False,
                                stop=(i == 2),
                                skip_group_check=True,
                            )

                # PSUM -> SBUF (single DVE copy per chunk), then DMA out
                nc.vector.tensor_copy(out=to[:, : NG * W], in_=ps[:, : NG * W])
                for g in range(NG):
                    G = G0 + g
                    if G == NGRP - 1:
                        nc.sync.dma_start(
                            out=_dram_ap(
                                yout,
                                [[W, 8], [_HW, OC], [1, W]],
                                s * OC * _HW + 504 * W,
                            ),
                            in_=to[0:64, g * W : (g + 1) * W],
                        )
                    else:
                        nc.sync.dma_start(
                            out=_dram_ap(
                                yout,
                                [[W, TOUT], [_HW, OC], [1, W]],
                                s * OC * _HW + 14 * G * W,
                            ),
                            in_=to[:, g * W : (g + 1) * W],
                        )


WP = W + 2  # host-padded row width (zero col at each edge; f32r needs even N)
_HWP = H * WP
_SAMP_P = IC * _HWP


def _emit_conv_pair(nc, tc, xin, yout, c32):
    """Row-pair layout: partition = (rq*8+ic) in [0,64), each partition's
    tile slice holds TWO consecutive (width-padded) image rows (~4KB
    contiguous DMA packets). 6 f32r tap matmuls (3 dx x 2 parity) + bias
    per group; x is host-padded to width 514 so every tap is N=512."""
    f32 = mybir.dt.float32
    f32r = mybir.dt.float32r
    import contextlib

    with contextlib.ExitStack() as ctx:
        consts = ctx.enter_context(tc.tile_pool(name="consts", bufs=1))
        inp = ctx.enter_context(tc.tile_pool(name="inp", bufs=4))
        outp = ctx.enter_context(tc.tile_pool(name="outp", bufs=3))
        psum = ctx.enter_context(tc.tile_pool(name="psum", bufs=2, space="PSUM"))

        c32_sb = consts.tile([128, _CONST_COLS], f32r)
        nc.sync.dma_start(out=c32_sb[:], in_=c32[:])
        ones_ap = c32_sb[0:1, _ONES_OFF : _ONES_OFF + W]

        for s in range(SPC):
            for G0, NG in CHUNKS:
                ti = inp.tile([64, 4, 2, WP], f32r, tag="ti")
                for g in range(NG):
                    G = G0 + g
                    if G == 0:
                        nc.vector.memset(ti[0:8, g, 0:1, :].bitcast(f32), 0.0)  # row -1
                        nc.sync.dma_start(
                            out=ti[0:8, g, 1:2, :],
                            in_=_dram_ap(xin, [[_HWP, IC], [1, WP]], s * _SAMP_P),
                        )
                        nc.sync.dma_start(
                            out=ti[8:64, g, :, :],
                            in_=_dram_ap(
                                xin,
                                [[2 * WP, 7], [_HWP, IC], [1, 2 * WP]],
                                s * _SAMP_P + WP,
                            ),
                        )
                    elif G == NGRP - 1:
                        nc.vector.memset(ti[32:64, g, :, :].bitcast(f32), 0.0)
                        nc.sync.dma_start(
                            out=ti[0:32, g, :, :],
                            in_=_dram_ap(
                                xin,
                                [[2 * WP, 4], [_HWP, IC], [1, 2 * WP]],
                                s * _SAMP_P + 503 * WP,
                            ),
                        )
                        nc.sync.dma_start(
                            out=ti[32:40, g, 0:1, :],
                            in_=_dram_ap(
                                xin, [[_HWP, IC], [1, WP]], s * _SAMP_P + 511 * WP
                            ),
                        )
                    else:
                        nc.sync.dma_start(
                            out=ti[:, g, :, :],
                            in_=_dram_ap(
                                xin,
                                [[2 * WP, 8], [_HWP, IC], [1, 2 * WP]],
                                s * _SAMP_P + (14 * G - 1) * WP,
                            ),
                        )

                to = outp.tile([M_PART, 4 * W], f32, tag="to")
                ps = psum.tile([M_PART, 4 * W], f32)

                for g in range(NG):
                    G = G0 + g
                    v = 1 if G == 0 else (2 if G == NGRP - 1 else 0)
                    pcol = g * W
                    nc.tensor.matmul(
                        ps[:, pcol : pcol + W],
                        c32_sb[0:1, _BIAS_OFF + s * M_PART : _BIAS_OFF + (s + 1) * M_PART],
                        ones_ap,
                        start=True,
                        stop=False,
                        skip_group_check=True,
                    )
                    for i, (dx, p) in enumerate(
                        [(dx, p) for dx in range(3) for p in range(2)]
                    ):
                        col = _a_col_pair(s, v, dx, p)
                        nc.tensor.matmul(
                            ps[:, pcol : pcol + W],
                            c32_sb[0:64, col : col + M_PART],
                            ti[:, g, p, dx : dx + W],
                            start=False,
                            stop=(i == 5),
                            skip_group_check=True,
                        )

                nc.vector.tensor_copy(out=to[:, : NG * W], in_=ps[:, : NG * W])
                for g in range(NG):
                    G = G0 + g
                    if G == NGRP - 1:
                        nc.gpsimd.dma_start(
                            out=_dram_ap(
                                yout, [[W, 8], [_HW, OC], [1, W]],
                                s * OC * _HW + 504 * W,
                            ),
                            in_=to[0:64, g * W : (g + 1) * W],
                        )
                    else:
                        nc.gpsimd.dma_start(
                            out=_dram_ap(
                                yout, [[W, TOUT], [_HW, OC], [1, W]],
                                s * OC * _HW + 14 * G * W,
                            ),
                            in_=to[:, g * W : (g + 1) * W],
                        )


def _emit_conv_f32r(nc, tc, xin, yout, c32):
    """K=128 (16 rows x 8 ic) layout, f32r taps: 3 matmuls per group,
    bias folded into the DVE PSUM->SBUF copy (tensor_scalar_add). x is
    host-padded to width 514 so every tap is N=512 (f32r needs even N)."""
    f32 = mybir.dt.float32
    f32r = mybir.dt.float32r
    import contextlib

    with contextlib.ExitStack() as ctx:
        consts = ctx.enter_context(tc.tile_pool(name="consts", bufs=1))
        inp = ctx.enter_context(tc.tile_pool(name="inp", bufs=6))
        outp = ctx.enter_context(tc.tile_pool(name="outp", bufs=4))
        psum = ctx.enter_context(tc.tile_pool(name="psum", bufs=2, space="PSUM"))

        c32_sb = consts.tile([128, _CONST_COLS], f32r)
        nc.sync.dma_start(out=c32_sb[:], in_=c32[:])
        bias_s = [
            c32_sb[0:M_PART, _BVEC_OFF + s : _BVEC_OFF + s + 1].bitcast(f32)
            for s in range(SPC)
        ]

        ci = 0  # alternate in-DMA issue between sync and scalar queues
        for s in range(SPC):
            for G0, NG in CHUNKS:
                ti = inp.tile([128, 4, WP], f32r, tag="ti")
                for g in range(NG):
                    G = G0 + g
                    eng = nc.sync if ci % 2 == 0 else nc.scalar
                    ci += 1
                    if G == 0:
                        nc.vector.memset(ti[0:32, g : g + 1, :].bitcast(f32), 0.0)
                        eng.dma_start(
                            out=ti[8:128, g : g + 1, :],
                            in_=_dram_ap(
                                xin, [[WP, 15], [_HWP, IC], [1, WP]], s * _SAMP_P
                            ),
                        )
                    elif G == NGRP - 1:
                        nc.vector.memset(ti[64:128, g : g + 1, :].bitcast(f32), 0.0)
                        eng.dma_start(
                            out=ti[0:72, g : g + 1, :],
                            in_=_dram_ap(
                                xin,
                                [[WP, 9], [_HWP, IC], [1, WP]],
                                s * _SAMP_P + 503 * WP,
                            ),
                        )
                    else:
                        eng.dma_start(
                            out=ti[:, g : g + 1, :],
                            in_=_dram_ap(
                                xin,
                                [[WP, TIN], [_HWP, IC], [1, WP]],
                                s * _SAMP_P + (14 * G - 1) * WP,
                            ),
                        )

                to = outp.tile([M_PART, 4 * W], f32, tag="to")
                ps = psum.tile([M_PART, 4 * W], f32)

                for g in range(NG):
                    G = G0 + g
                    v = 1 if G == 0 else (2 if G == NGRP - 1 else 0)
                    pcol = g * W
                    for dx in range(3):
                        col = _a_col(s, v, dx)
                        nc.tensor.matmul(
                            ps[:, pcol : pcol + W],
                            c32_sb[:, col : col + M_PART],
                            ti[:, g, dx : dx + W],
                            start=(dx == 0),
                            stop=(dx == 2),
                            skip_group_check=True,
                        )
                nc.vector.tensor_scalar_add(
                    out=to[:, : NG * W], in0=ps[:, : NG * W], scalar1=bias_s[s]
                )
                for g in range(NG):
                    G = G0 + g
                    pcol = g * W
                    if G == NGRP - 1:
                        nc.gpsimd.dma_start(
                            out=_dram_ap(
                                yout, [[W, 8], [_HW, OC], [1, W]],
                                s * OC * _HW + 504 * W,
                            ),
                            in_=to[0:64, pcol : pcol + W],
                        )
                    else:
                        nc.gpsimd.dma_start(
                            out=_dram_ap(
                                yout, [[W, TOUT], [_HW, OC], [1, W]],
                                s * OC * _HW + 14 * G * W,
                            ),
                            in_=to[:, pcol : pcol + W],
                        )


_HP = 14 * NGRP + 2  # 520 row-padded height (row -1 zero, rows 512+ zero)
_NCH = len(CHUNKS)  # 10


def _emit_conv_bf16(nc, tc, xin, yout, ca, cb):
    """Pure-bf16 conv. rel-err budget is 2e-2; bf16 rounding gives ~2e-3.

    x in DRAM is host-prepped: [SPC, HP, IC, WP] bf16, row/col
    zero-padded and (H, IC)-transposed so each 16-row group slab is one
    fully contiguous 131 KB block -> large aggregated DMA packets. Each
    group is 3 accumulating tap matmuls (K=128 = 16 rows x 8 ic,
    M=112 = 14 out rows x 8 oc, N=512); no edge variants, no bias
    matmul. Bias rides the PSUM->SBUF copy (tensor_scalar_add,
    alternating DVE/Act). Output goes out as bf16 in SBUF-tile layout
    [SPC, chunk, 112, 2048] (4 KB contiguous lines) and is unscrambled
    on host."""
    f32 = mybir.dt.float32
    bf16 = mybir.dt.bfloat16
    import contextlib

    with contextlib.ExitStack() as ctx:
        consts = ctx.enter_context(tc.tile_pool(name="consts", bufs=1))
        inp = ctx.enter_context(tc.tile_pool(name="inp", bufs=4))
        outp = ctx.enter_context(tc.tile_pool(name="outp", bufs=4))
        psum = ctx.enter_context(tc.tile_pool(name="psum", bufs=2, space="PSUM"))

        ca_sb = consts.tile([128, _CA_COLS], bf16)
        nc.sync.dma_start(out=ca_sb[:], in_=ca[:])
        cb_sb = consts.tile([M_PART, SPC], f32)
        nc.gpsimd.dma_start(out=cb_sb[:], in_=cb[:])

        ci = 0
        for s in range(SPC):
            for G0, NG in CHUNKS:
                ti = inp.tile([128, 4, WP], bf16, tag="ti")
                for g in range(NG):
                    eng = nc.sync if (ci + g) % 2 == 0 else nc.gpsimd
                    eng.dma_start(
                        out=ti[:, g : g + 1, :],
                        in_=_dram_ap(
                            xin,
                            [[WP, 128], [1, WP]],
                            (s * _HP + 14 * (G0 + g)) * IC * WP,
                        ),
                    )
                ps = psum.tile([M_PART, 4 * W], f32)
                for dx in range(3):
                    acol = (s * 3 + dx) * M_PART
                    for g in range(NG):
                        nc.tensor.matmul(
                            ps[:, g * W : (g + 1) * W],
                            ca_sb[:, acol : acol + M_PART],
                            ti[:, g, dx : dx + W],
                            start=(dx == 0),
                            stop=(dx == 2),
                            skip_group_check=True,
                        )
                to = outp.tile([M_PART, 4 * W], bf16, tag="to")
                if ci % 2 == 0:
                    nc.vector.tensor_scalar_add(
                        out=to[:, : NG * W],
                        in0=ps[:, : NG * W],
                        scalar1=cb_sb[0:M_PART, s : s + 1],
                    )
                else:
                    nc.scalar.activation(
                        out=to[:, : NG * W],
                        in_=ps[:, : NG * W],
                        func=mybir.ActivationFunctionType.Identity,
                        bias=cb_sb[0:M_PART, s : s + 1],
                    )
                oeng = nc.gpsimd if ci % 2 == 0 else nc.sync
                oeng.dma_start(
                    out=_dram_ap(
                        yout,
                        [[4 * W, M_PART], [1, NG * W]],
                        (s * _NCH + G0 // 4) * M_PART * 4 * W,
                    ),
                    in_=to[:, : NG * W],
                )
                ci += 1


def _emit_conv_s4(nc, tc, xin, yout, ca, cb):
    """4-sample-interleaved bf16 conv, one group per chunk.

    x in DRAM: [HP, IC, SPC, W] bf16 (rows zero-padded, samples
    interleaved within each (row, ic) line) -> every partition line is
    exactly 4096 B and each 16-row group slab is one 512 KB contiguous
    DMA; packets aggregate to aligned 4 KB. No width padding: the
    center tap (dx=1) runs full N=512 with start=True, the dx=0/2 taps
    run N=511 with clipped psum/rhs ranges (zero-pad semantics).
    Output: [NGRP, 112, SPC*W] bf16, unscrambled on host."""
    f32 = mybir.dt.float32
    bf16 = mybir.dt.bfloat16
    import contextlib

    SW = SPC * W  # 2048 cols per line

    with contextlib.ExitStack() as ctx:
        consts = ctx.enter_context(tc.tile_pool(name="consts", bufs=1))
        inp = ctx.enter_context(tc.tile_pool(name="inp", bufs=6))
        outp = ctx.enter_context(tc.tile_pool(name="outp", bufs=6))
        psum = ctx.enter_context(tc.tile_pool(name="psum", bufs=8, space="PSUM"))

        # slab 0 first: it gates the first matmul; consts are smaller and
        # ride the other rings concurrently.
        ti0 = inp.tile([128, SW], bf16, tag="ti")
        ca_sb = consts.tile([128, _CA_COLS], bf16)
        cb_sb = consts.tile([M_PART, SPC], f32)
        # tiny cb first: its 16-way fan-out spins up every DMA engine
        # before the large first-load transfers land on them
        nc.sync.dma_start(out=cb_sb[:], in_=cb[:])
        # balance the ~856KB of first loads across the 3 rings:
        # sync q1+caA (300KB), gpsimd q2+caB (300KB), scalar q3+q4 (256KB)
        for qi, qeng in enumerate((nc.sync, nc.gpsimd, nc.scalar, nc.scalar)):
            qeng.dma_start(
                out=ti0[32 * qi : 32 * qi + 32, :],
                in_=_dram_ap(xin, [[SW, 32], [1, SW]], 32 * qi * SW),
            )
        nc.sync.dma_start(
            out=ca_sb[0:64, :],
            in_=_dram_ap(ca, [[_CA_COLS, 64], [1, _CA_COLS]], 0),
        )
        nc.gpsimd.dma_start(
            out=ca_sb[64:128, :],
            in_=_dram_ap(ca, [[_CA_COLS, 64], [1, _CA_COLS]], 64 * _CA_COLS),
        )

        # (dx, psum col offset, rhs col offset, N)
        TAPS = [(1, 0, 0, W), (0, 1, 0, W - 1), (2, 0, 1, W - 1)]

        for G in range(NGRP):
            if G == 0:
                ti = ti0
            else:
                ti = inp.tile([128, SW], bf16, tag="ti")
                ieng = (nc.sync, nc.gpsimd) if G % 2 == 0 else (nc.gpsimd, nc.sync)
                ieng[0].dma_start(
                    out=ti[0:64, :],
                    in_=_dram_ap(xin, [[SW, 64], [1, SW]], 14 * G * IC * SW),
                )
                ieng[1].dma_start(
                    out=ti[64:128, :],
                    in_=_dram_ap(xin, [[SW, 64], [1, SW]], (14 * G * IC + 64) * SW),
                )
            for s in range(SPC):
                ps = psum.tile([M_PART, W], f32, tag="ps")
                for i, (dx, po, ro, N) in enumerate(TAPS):
                    acol = (s * 3 + dx) * M_PART
                    nc.tensor.matmul(
                        ps[:, po : po + N],
                        ca_sb[:, acol : acol + M_PART],
                        ti[:, s * W + ro : s * W + ro + N],
                        start=(i == 0),
                        stop=(i == 2),
                        skip_group_check=True,
                    )
                if s == 0:
                    to = outp.tile([M_PART, SW], bf16, tag="to")
                cp = 8 * OC if G == NGRP - 1 else M_PART
                if s % 2 == G % 2:
                    nc.vector.tensor_scalar_add(
                        out=to[0:cp, s * W : (s + 1) * W],
                        in0=ps[0:cp, :],
                        scalar1=cb_sb[0:cp, s : s + 1],
                    )
                else:
                    nc.scalar.activation(
                        out=to[0:cp, s * W : (s + 1) * W],
                        in_=ps[0:cp, :],
                        func=mybir.ActivationFunctionType.Identity,
                        bias=cb_sb[0:cp, s : s + 1],
                    )
            if G == NGRP - 1:
                # final chunk: per-sample DMAs overlap the copies, pulling
                # the last transfer off the tail
                for s in range(SPC):
                    eng = (nc.scalar, nc.sync, nc.scalar, nc.sync)[s]
                    eng.dma_start(
                        out=_dram_ap(
                            yout, [[SW, 8 * OC], [1, W]], G * M_PART * SW + s * W
                        ),
                        in_=to[0 : 8 * OC, s * W : (s + 1) * W],
                    )
            else:
                nc.scalar.dma_start(
                    out=_dram_ap(yout, [[SW, M_PART], [1, SW]], G * M_PART * SW),
                    in_=to[:],
                )


_NC_CACHE = {}


def _get_nc():
    if MODE in _NC_CACHE:
        return _NC_CACHE[MODE]
    f32 = mybir.dt.float32
    bf16 = mybir.dt.bfloat16
    nc = bass.Bass("TRN2", target_bir_lowering=False, debug=False, num_devices=NCORES)
    if MODE == "s4":
        SW = SPC * W
        xin = nc.declare_dram_parameter("x", [_HP, IC, SPC, W], bf16, isOutput=False)
        ca = nc.declare_dram_parameter("ca", [128, _CA_COLS], bf16, isOutput=False)
        cb = nc.declare_dram_parameter("cb", [M_PART, SPC], f32, isOutput=False)
        yout = nc.declare_dram_parameter(
            "y", [NGRP, M_PART, SW], bf16, isOutput=True
        )
        with TileContext(nc) as tc:
            _emit_conv_s4(nc, tc, xin, yout, ca, cb)
        _legalize_waits(nc)
        _NC_CACHE[MODE] = nc
        return nc
    if MODE == "bf16":
        xin = nc.declare_dram_parameter("x", [SPC, _HP, IC, WP], bf16, isOutput=False)
        ca = nc.declare_dram_parameter("ca", [128, _CA_COLS], bf16, isOutput=False)
        cb = nc.declare_dram_parameter("cb", [M_PART, SPC], f32, isOutput=False)
        yout = nc.declare_dram_parameter(
            "y", [SPC, _NCH, M_PART, 4 * W], bf16, isOutput=True
        )
        with TileContext(nc) as tc:
            _emit_conv_bf16(nc, tc, xin, yout, ca, cb)
        _legalize_waits(nc)
        _NC_CACHE[MODE] = nc
        return nc
    cdt = mybir.dt.float32r if MODE in ("split", "f32r", "pair") else f32
    xdt = cdt if MODE in ("f32r", "f32", "pair") else f32
    xshape = [SPC, IC, H, WP] if MODE in ("pair", "f32r") else [SPC, IC, H, W]
    xin = nc.declare_dram_parameter("x", xshape, xdt, isOutput=False)
    c32 = nc.declare_dram_parameter("c32", [128, _CONST_COLS], cdt, isOutput=False)
    cbf = None
    if MODE == "split":
        cbf = nc.declare_dram_parameter(
            "cbf", [128, 2 * _A_COLS], mybir.dt.bfloat16, isOutput=False
        )
    yout = nc.declare_dram_parameter("y", [SPC, OC, H, W], f32, isOutput=True)
    with TileContext(nc) as tc:
        if MODE == "pair":
            _emit_conv_pair(nc, tc, xin, yout, c32)
        elif MODE == "f32r":
            _emit_conv_f32r(nc, tc, xin, yout, c32)
        else:
            _emit_conv(nc, tc, xin, yout, c32, cbf)
    _legalize_waits(nc)
    _NC_CACHE[MODE] = nc
    return nc


def _build_stationaries_bf16(w):
    """w: [B, OC, IC, 3, 3] -> A: [B, 3 dx, 128, 112] bf16, no variants
    (row zero-padding lives in the DRAM x image instead)."""
    import ml_dtypes

    A = np.zeros((B, 3, 128, M_PART), np.float32)
    for dy in range(3):
        blk = w[:, :, :, dy, :].transpose(0, 3, 2, 1)  # [B, dx, IC, OC]
        for t in range(TOUT):
            A[:, :, (t + dy) * 8 : (t + dy) * 8 + 8, t * 8 : t * 8 + 8] = blk
    return A.astype(ml_dtypes.bfloat16)


def _prepare_bf16(q, x, wm1, bm1, wm2, bm2, wt, bt, wb, bb):
    import ml_dtypes

    q, x = np.asarray(q, np.float32), np.asarray(x, np.float32)
    args = [np.asarray(a, np.float32) for a in (wm1, bm1, wm2, bm2, wt, bt, wb, bb)]
    w, b = _manifold(q, *args)
    A = _build_stationaries_bf16(w)  # [B, 3, 128, 112] bf16

    xp = np.zeros((B, _HP, IC, WP), ml_dtypes.bfloat16)
    xp[:, 1 : 1 + H, :, 1 : 1 + W] = x.transpose(0, 2, 1, 3).astype(ml_dtypes.bfloat16)

    in_maps = []
    for k in range(NCORES):
        sl = slice(k * SPC, (k + 1) * SPC)
        # ca[k_part, (s*3+dx)*112 + m]
        ca = np.ascontiguousarray(
            A[sl].transpose(2, 0, 1, 3).reshape(128, _CA_COLS)
        )
        cb = np.empty((M_PART, SPC), np.float32)
        for s in range(SPC):
            cb[:, s] = np.tile(b[sl][s], TOUT)
        in_maps.append({"x": np.ascontiguousarray(xp[sl]), "ca": ca, "cb": cb})
    return _get_nc(), in_maps


def _unscramble_bf16(yd):
    """yd: [SPC, 10, 112, 2048] bf16 (one core) -> [SPC, OC, H, W] f32."""
    full = yd[:, : _NCH - 1].astype(np.float32)
    full = full.reshape(SPC, _NCH - 1, TOUT, OC, 4, W)  # [s, c, ro, oc, g, w]
    full = full.transpose(0, 3, 1, 4, 2, 5).reshape(SPC, OC, (_NCH - 1) * 4 * TOUT, W)
    last = yd[:, _NCH - 1, : 8 * OC, :W].astype(np.float32)
    last = last.reshape(SPC, 8, OC, W).transpose(0, 2, 1, 3)  # [s, oc, ro, w]
    return np.concatenate([full, last], axis=2)


def _prepare_s4(q, x, wm1, bm1, wm2, bm2, wt, bt, wb, bb):
    import ml_dtypes

    q, x = np.asarray(q, np.float32), np.asarray(x, np.float32)
    args = [np.asarray(a, np.float32) for a in (wm1, bm1, wm2, bm2, wt, bt, wb, bb)]
    w, b = _manifold(q, *args)
    A = _build_stationaries_bf16(w)  # [B, 3, 128, 112] bf16

    xb = x.astype(ml_dtypes.bfloat16)
    in_maps = []
    for k in range(NCORES):
        sl = slice(k * SPC, (k + 1) * SPC)
        ca = np.ascontiguousarray(A[sl].transpose(2, 0, 1, 3).reshape(128, _CA_COLS))
        cb = np.empty((M_PART, SPC), np.float32)
        for s in range(SPC):
            cb[:, s] = np.tile(b[sl][s], TOUT)
        # xq[row+1, ic, s, w] = x[s, ic, row, w]; rows 0 and 513.. are zero
        xq = np.zeros((_HP, IC, SPC, W), ml_dtypes.bfloat16)
        xq[1 : 1 + H] = xb[sl].transpose(2, 1, 0, 3)
        in_maps.append({"x": xq, "ca": ca, "cb": cb})
    return _get_nc(), in_maps


def _unscramble_s4(yd):
    """yd: [NGRP, 112, SPC*W] bf16 (one core) -> [SPC, OC, H, W] f32."""
    full = yd[: NGRP - 1].astype(np.float32)
    full = full.reshape(NGRP - 1, TOUT, OC, SPC, W)  # [G, ro, oc, s, w]
    full = full.transpose(3, 2, 0, 1, 4).reshape(SPC, OC, (NGRP - 1) * TOUT, W)
    last = yd[NGRP - 1, : 8 * OC].astype(np.float32)
    last = last.reshape(8, OC, SPC, W).transpose(2, 1, 0, 3)  # [s, oc, ro, w]
    return np.concatenate([full, last], axis=2)


def _prepare(q, x, wm1, bm1, wm2, bm2, wt, bt, wb, bb):
    if MODE == "s4":
        return _prepare_s4(q, x, wm1, bm1, wm2, bm2, wt, bt, wb, bb)
    if MODE == "bf16":
        return _prepare_bf16(q, x, wm1, bm1, wm2, bm2, wt, bt, wb, bb)
    q, x = np.asarray(q, np.float32), np.asarray(x, np.float32)
    args = [np.asarray(a, np.float32) for a in (wm1, bm1, wm2, bm2, wt, bt, wb, bb)]
    w, b = _manifold(q, *args)
    A = _build_stationaries_pair(w) if MODE == "pair" else _build_stationaries(w)

    if MODE in ("pair", "f32r"):
        xp = np.zeros((B, IC, H, WP), np.float32)
        xp[:, :, :, 1 : 1 + W] = x
        x = xp

    in_maps = []
    for k in range(NCORES):
        sl = slice(k * SPC, (k + 1) * SPC)
        C = _build_consts(A[sl], b[sl])
        im = {"x": np.ascontiguousarray(x[sl]), "c32": C}
        if MODE == "split":
            import ml_dtypes

            Ablock = C[:, :_A_COLS].astype(np.float32)
            Ah = Ablock.astype(ml_dtypes.bfloat16)
            Al = (Ablock - Ah.astype(np.float32)).astype(ml_dtypes.bfloat16)
            im["cbf"] = np.concatenate([Ah, Al], axis=1)
        in_maps.append(im)
    return _get_nc(), in_maps


def kernel(q, x, wm1, bm1, wm2, bm2, wt, bt, wb, bb):
    nc, in_maps = _prepare(q, x, wm1, bm1, wm2, bm2, wt, bt, wb, bb)
    br = run_bass_kernel_spmd(nc, in_maps, list(range(NCORES)))
    if MODE == "s4":
        return np.concatenate(
            [_unscramble_s4(np.asarray(r["y"])) for r in br.results], axis=0
        )
    if MODE == "bf16":
        return np.concatenate(
            [_unscramble_bf16(np.asarray(r["y"])) for r in br.results], axis=0
        )
    return np.concatenate([r["y"] for r in br.results], axis=0)



# revision 36
# speedup vs baseline: 1.0261x; 1.0261x over previous
"""ConvolutionalFilterManifold Trainium2 kernel.

Reference: a tiny "manifold" MLP maps q[B,1,8,8] -> per-sample 3x3 conv
filters w[B,8,8,3,3] and biases b[B,8]; the heavy op is a per-sample
conv2d over x[B,8,512,512] (pad 1, stride 1) -> y[B,8,512,512].

Strategy: manifold on host (tiny, exact); conv on 8 NeuronCores with
pure batch data-parallelism (4 samples/core). Default MODE "s4" is
pure bf16 (rel err ~2.9e-3 against the 2e-2 gate):

- Per output row-group G of TOUT=14 rows, stationary A_dx[(ri*8+ic),
  (ro*8+oc)] = w[oc, ic, ri-ro, dx] (banded block-Toeplitz, K=128 =
  16 input rows x 8 in-chans, M=112 = 14 out rows x 8 out-chans);
  3 dx taps = 3 PSUM-accumulating bf16 matmuls per (group, sample).
  Center tap runs full N=512 with start=True; dx=0/2 use clipped
  psum/rhs ranges (zero width-padding semantics). Bias rides the
  PSUM->SBUF copy (DVE tensor_scalar_add / Act Identity+bias).
- DMA layout is the key: x is host-prepped to [HP=520, IC, SPC, W]
  bf16 with the 4 samples interleaved inside each (row, ic) line, so
  every SBUF partition line is exactly 4096 B (the DMA packet sweet
  spot: ~23 GB/s/engine x 16 engines) and each 16-row group slab is
  one fully contiguous 512 KB read, split into two 64-line halves on
  separate queue rings (sync/gpsimd). Output goes out as bf16 in SBUF
  tile layout [NGRP, 112, SPC*W] (4 KB lines, scalar-ring DMA) and is
  unscrambled + upcast on host. Row zero-padding is baked into the
  DRAM image (rows 0 and 513+), so all 37 slabs are uniform: no edge
  variants, no memsets, no bias matmul.

Hardware constraint discovered empirically: every TPB instruction has
ONE sync-wait slot (bf16 matmuls get 2 via the LDW+MM split; 4-byte
self-loading matmuls get just 1). The emission order below keeps every
instruction's Tile-assigned wait count within its slots, and the
TileContext drain is patched to spread its per-proc waits over nops.
"""

import os
import re
import sys

sys.path.insert(0, "/opt/trn_rl_repo")

import numpy as np  # noqa: E402

import bass_rust  # noqa: E402
import concourse.bass as bass  # noqa: E402
import concourse.mybir as mybir  # noqa: E402
from concourse.bass_utils import run_bass_kernel_spmd  # noqa: E402
from concourse.tile import TileContext  # noqa: E402
from concourse.vector_clock import ScopedClock  # noqa: E402

B, IC, OC = 32, 8, 8
H = W = 512
NCORES = 8
SPC = B // NCORES  # samples per core
TOUT = 14  # output rows per group
TIN = 16  # input rows per group (TOUT + 2)
NGRP = 37  # 36 full groups + one 8-row group
M_PART = OC * TOUT  # 112 psum partitions
# (group-start, n-groups) chunks; 4 groups -> 4 PSUM banks, x2 bufs = 8
CHUNKS = [(g, min(4, NGRP - g)) for g in range(0, NGRP, 4)]

MODE = os.environ.get("CFM_MODE", "s4")  # s4 | bf16 | split | f32r | f32 | pair

_ORIG_DRAIN = TileContext._drain_and_barrier


def _patched_drain_and_barrier(self, tick_clock, wait_clock):
    gc = tick_clock.global_clock
    vals = [int(v) for v in re.findall(r"-?\d+", repr(gc))]
    for i, v in enumerate(vals):
        if v > 0:
            sub = [0] * len(vals)
            sub[i] = v
            nop = self.nc.sync.nop(nofuse=True)
            wait_clock.add_sem_waits(
                nop.ins, ScopedClock({None: bass_rust.VectorClock(sub)})
            )
    self.nc.sync.drain()
    self.nc.all_engine_barrier()
    assert self.sems is not None
    popped = self.nc._tile_sem_poison_stack.pop()
    assert popped is self._sem_poison
    self.nc.clear_and_free_semaphores(list(self.sems.allocated().values()))
    self.nc.all_engine_barrier()


TileContext._drain_and_barrier = _patched_drain_and_barrier


def _legalize_waits(nc):
    """Every TPB instruction encodes at most ONE sync wait. Tile can
    attach several (multi-queue DMA producers, tile-granular WAR
    fan-ins). Hoist the excess onto same-engine InstNoOps inserted
    right before the instruction — the engine then blocks on the same
    sem set, just sequentially."""
    for fn in nc.m.functions:
        for bb in fn.blocks:
            out, changed = [], False
            for inst in bb.instructions:
                si = inst.sync_info
                if si is not None and len(si.on_wait) > 1:
                    waits = list(si.on_wait)
                    for w in waits[:-1]:
                        out.append(
                            mybir.InstNoOp(
                                name=nc.get_next_instruction_name(),
                                engine=inst.engine,
                                bass_nofuse=True,
                                sync_info=mybir.SyncInfo(on_wait=[w], on_update=[]),
                            )
                        )
                    inst.sync_info = mybir.SyncInfo(
                        on_wait=waits[-1:], on_update=list(si.on_update)
                    )
                    changed = True
                out.append(inst)
            if changed:
                bb.instructions = out


def _dram_ap(t, ap_list, offset):
    a = t[:].copy()
    a.ap = bass_rust.VecI64Pair(ap_list)
    a.offset = offset
    return a


def _manifold(q, wm1, bm1, wm2, bm2, wt, bt, wb, bb):
    m = np.einsum("bihw,cihw->bc", q, wm1) + bm1
    m = np.where(m > 0, m, np.float32(0.01) * m).astype(np.float32)
    m = m @ wm2.T + bm2
    m = np.where(m > 0, m, np.float32(0.01) * m).astype(np.float32)
    w = np.einsum("bc,cokl->bokl", m, wt) + bt[None, :, None, None]
    w = w.reshape(B, OC, IC, 3, 3).astype(np.float32)
    b = (m @ wb.T + bb).astype(np.float32)
    return w, b


def _build_stationaries(w):
    """w: [B, OC, IC, 3, 3] -> A: [B, 3 variants, 3 dx, 128, 112] f32.

    A[s, v, dx, ri*8+ic, ro*8+oc] = w[s, oc, ic, ri-ro, dx] for
    0 <= ri-ro <= 2 else 0. Variant 1 zeroes rows ri=0 (G=0, row -1);
    variant 2 zeroes rows ri>=9 (G=36, rows >=512)."""
    A = np.zeros((B, 3, 3, 128, M_PART), np.float32)
    ro = np.arange(TOUT)
    for dy in range(3):
        ri = ro + dy  # 14 values in [dy, dy+13]
        # block [ri*8 + ic, ro*8 + oc] = w[:, oc, ic, dy, dx]
        # use advanced indexing over (ro, ic, oc)
        blk = w[:, :, :, dy, :]  # [B, OC, IC, 3dx]
        for t in range(TOUT):
            A[:, 0, :, (t + dy) * 8 : (t + dy) * 8 + 8, t * 8 : t * 8 + 8] = (
                blk.transpose(0, 3, 2, 1)  # [B, dx, IC, OC]
            )
    A[:, 1] = A[:, 0]
    A[:, 1, :, 0:8, :] = 0.0
    A[:, 2] = A[:, 0]
    A[:, 2, :, 72:, :] = 0.0
    return A


def _build_stationaries_pair(w):
    """Row-pair variant: A[B, 3v, 3dx, 2parity, 64, 112] with
    A[s,v,dx,p, rq*8+ic, ro*8+oc] = w[s,oc,ic, 2rq+p-ro, dx] for
    0 <= 2rq+p-ro <= 2. Variant 1 zeroes input row index ri=2rq+p == 0;
    variant 2 zeroes ri >= 9."""
    A = np.zeros((B, 3, 3, 2, 64, M_PART), np.float32)
    for p in range(2):
        for rq in range(8):
            ri = 2 * rq + p
            for ro in range(TOUT):
                dy = ri - ro
                if 0 <= dy <= 2:
                    A[:, 0, :, p, rq * 8 : rq * 8 + 8, ro * 8 : ro * 8 + 8] = w[
                        :, :, :, dy, :
                    ].transpose(0, 3, 2, 1)
    A[:, 1] = A[:, 0]
    A[:, 1, :, 0, 0:8, :] = 0.0  # ri = 0
    A[:, 2] = A[:, 0]
    for p in range(2):
        for rq in range(8):
            if 2 * rq + p >= 9:
                A[:, 2, :, p, rq * 8 : rq * 8 + 8, :] = 0.0
    return A


if MODE == "pair":
    _A_COLS = SPC * 3 * 3 * 2 * M_PART  # 8064
else:
    _A_COLS = SPC * 3 * 3 * M_PART  # 4032
_CA_COLS = SPC * 3 * M_PART  # 1344 (bf16 mode: no edge variants)
_ONES_OFF = _A_COLS
_BIAS_OFF = _A_COLS + W
_BVEC_OFF = _A_COLS  # f32r mode: per-partition bias vectors instead of ones/row
_CONST_COLS = _A_COLS + SPC if MODE == "f32r" else _BIAS_OFF + SPC * M_PART


def _build_consts(A_core, b_core):
    """Pack per-core consts into one [128, _CONST_COLS] f32 image."""
    C = np.zeros((128, _CONST_COLS), np.float32)
    if MODE == "pair":
        C[:64, :_A_COLS] = A_core.transpose(4, 0, 1, 2, 3, 5).reshape(64, _A_COLS)
    else:
        C[:, :_A_COLS] = A_core.transpose(3, 0, 1, 2, 4).reshape(128, _A_COLS)
    if MODE == "f32r":
        # bias as [112, SPC] per-partition column vectors (DVE adds them)
        for s in range(SPC):
            C[0:M_PART, _BVEC_OFF + s] = np.tile(b_core[s], TOUT)
    else:
        C[0, _ONES_OFF : _ONES_OFF + W] = 1.0
        bias_block = np.repeat(b_core[:, None, :], TOUT, axis=1)  # [SPC, 14, 8]
        C[0, _BIAS_OFF :] = bias_block.reshape(-1)
    return C


def _a_col(s, v, dx):
    return ((s * 3 + v) * 3 + dx) * M_PART


def _a_col_pair(s, v, dx, p):
    return ((((s * 3 + v) * 3 + dx) * 2) + p) * M_PART


_HW = H * W  # per-channel plane, elements
_SAMP = IC * _HW  # per-sample elements


def _emit_conv(nc, tc, xin, yout, c32, cbf):
    """Emit the per-core conv program.

    xin: DRAM [SPC, IC, H, W]; yout: DRAM [SPC, OC, H, W]
    c32: DRAM [128, _CONST_COLS] (f32r or f32 depending on mode)
    cbf: DRAM [128, 2*_A_COLS] bf16 (split mode only: A_hi | A_lo)
    """
    f32 = mybir.dt.float32
    bf16 = mybir.dt.bfloat16
    cdt = mybir.dt.float32r if MODE in ("split", "f32r") else f32
    xdt = cdt if MODE in ("f32r", "f32") else f32

    import contextlib

    with contextlib.ExitStack() as ctx:
        consts = ctx.enter_context(tc.tile_pool(name="consts", bufs=1))
        inp = ctx.enter_context(tc.tile_pool(name="inp", bufs=3))
        hlp = ctx.enter_context(tc.tile_pool(name="hlp", bufs=3))
        outp = ctx.enter_context(tc.tile_pool(name="outp", bufs=3))
        psum = ctx.enter_context(tc.tile_pool(name="psum", bufs=2, space="PSUM"))

        c32_sb = consts.tile([128, _CONST_COLS], cdt)
        nc.sync.dma_start(out=c32_sb[:], in_=c32[:])
        if MODE == "split":
            cbf_sb = consts.tile([128, 2 * _A_COLS], bf16)
            nc.sync.dma_start(out=cbf_sb[:], in_=cbf[:])

        ones_ap = c32_sb[0:1, _ONES_OFF : _ONES_OFF + W]

        for s in range(SPC):
            for G0, NG in CHUNKS:
                ti = inp.tile([128, 4, W], xdt, tag="ti")
                # ---- input DMAs, one per group (DMA APs max 3 dims)
                for g in range(NG):
                    G = G0 + g
                    if G == 0:
                        # zero the row(-1) slab: stale SBUF could hold NaNs
                        # and 0-weight x NaN still poisons the accumulation
                        nc.vector.memset(ti[0:32, 0:1, :], 0.0)
                        nc.sync.dma_start(
                            out=ti[8:128, 0:1, :],
                            in_=_dram_ap(xin, [[W, 15], [_HW, IC], [1, W]], s * _SAMP),
                        )
                    elif G == NGRP - 1:
                        nc.vector.memset(ti[64:128, g : g + 1, :], 0.0)
                        nc.sync.dma_start(
                            out=ti[0:72, g : g + 1, :],
                            in_=_dram_ap(
                                xin, [[W, 9], [_HW, IC], [1, W]], s * _SAMP + 503 * W
                            ),
                        )
                    else:
                        nc.sync.dma_start(
                            out=ti[:, g : g + 1, :],
                            in_=_dram_ap(
                                xin,
                                [[W, TIN], [_HW, IC], [1, W]],
                                s * _SAMP + (14 * G - 1) * W,
                            ),
                        )
                if MODE == "split":
                    th = hlp.tile([128, 4, W], bf16, tag="th")
                    tl = hlp.tile([128, 4, W], bf16, tag="tl")
                    for g in range(NG):
                        nc.vector.tensor_copy(
                            out=th[:, g : g + 1, :], in_=ti[:, g : g + 1, :]
                        )
                        nc.vector.tensor_sub(
                            out=tl[:, g : g + 1, :],
                            in0=ti[:, g : g + 1, :],
                            in1=th[:, g : g + 1, :],
                        )

                to = outp.tile([M_PART, 4 * W], f32, tag="to")
                ps = psum.tile([M_PART, 4 * W], f32)

                for g in range(NG):
                    G = G0 + g
                    v = 1 if G == 0 else (2 if G == NGRP - 1 else 0)
                    pcol = g * W
                    # bias pre-load: psum[:, :] = bias x ones  (start)
                    nc.tensor.matmul(
                        ps[:, pcol : pcol + W],
                        c32_sb[0:1, _BIAS_OFF + s * M_PART : _BIAS_OFF + (s + 1) * M_PART],
                        ones_ap,
                        start=True,
                        stop=False,
                        skip_group_check=True,
                    )
                    # dx taps: out cols [lo,hi) <- x cols [lo+dx-1, hi+dx-1)
                    taps = []
                    for dx in range(3):
                        lo = max(0, 1 - dx)
                        hi = W - max(0, dx - 1)
                        taps.append((dx, lo, hi))
                    if MODE == "split":
                        mm_ops = []
                        for dx, lo, hi in taps:
                            ah = cbf_sb[:, _a_col(s, v, dx) : _a_col(s, v, dx) + M_PART]
                            al = cbf_sb[
                                :,
                                _A_COLS + _a_col(s, v, dx) : _A_COLS
                                + _a_col(s, v, dx)
                                + M_PART,
                            ]
                            xh = th[:, g, lo + dx - 1 : hi + dx - 1]
                            xl = tl[:, g, lo + dx - 1 : hi + dx - 1]
                            mm_ops.append((ah, xh))
                            mm_ops.append((ah, xl))
                            mm_ops.append((al, xh))
                        for i, (a_ap, x_ap) in enumerate(mm_ops):
                            dx, lo, hi = taps[i // 3]
                            nc.tensor.matmul(
                                ps[:, pcol + lo : pcol + hi],
                                a_ap,
                                x_ap,
                                start=False,
                                stop=(i == len(mm_ops) - 1),
                                skip_group_check=True,
                            )
                    else:
                        for i, (dx, lo, hi) in enumerate(taps):
                            nc.tensor.matmul(
                                ps[:, pcol + lo : pcol + hi],
                                c32_sb[:, _a_col(s, v, dx) : _a_col(s, v, dx) + M_PART],
                                ti[:, g, lo + dx - 1 : hi + dx - 1],
                                start=False,
                                stop=(i == 2),
                                skip_group_check=True,
                            )

                # PSUM -> SBUF (single DVE copy per chunk), then DMA out
                nc.vector.tensor_copy(out=to[:, : NG * W], in_=ps[:, : NG * W])
                for g in range(NG):
                    G = G0 + g
                    if G == NGRP - 1:
                        nc.sync.dma_start(
                            out=_dram_ap(
                                yout,
                                [[W, 8], [_HW, OC], [1, W]],
                                s * OC * _HW + 504 * W,
                            ),
                            in_=to[0:64, g * W : (g + 1) * W],
                        )
                    else:
                        nc.sync.dma_start(
                            out=_dram_ap(
                                yout,
                                [[W, TOUT], [_HW, OC], [1, W]],
                                s * OC * _HW + 14 * G * W,
                            ),
                            in_=to[:, g * W : (g + 1) * W],
                        )


WP = W + 2  # host-padded row width (zero col at each edge; f32r needs even N)
_HWP = H * WP
_SAMP_P = IC * _HWP


def _emit_conv_pair(nc, tc, xin, yout, c32):
    """Row-pair layout: partition = (rq*8+ic) in [0,64), each partition's
    tile slice holds TWO consecutive (width-padded) image rows (~4KB
    contiguous DMA packets). 6 f32r tap matmuls (3 dx x 2 parity) + bias
    per group; x is host-padded to width 514 so every tap is N=512."""
    f32 = mybir.dt.float32
    f32r = mybir.dt.float32r
    import contextlib

    with contextlib.ExitStack() as ctx:
        consts = ctx.enter_context(tc.tile_pool(name="consts", bufs=1))
        inp = ctx.enter_context(tc.tile_pool(name="inp", bufs=4))
        outp = ctx.enter_context(tc.tile_pool(name="outp", bufs=3))
        psum = ctx.enter_context(tc.tile_pool(name="psum", bufs=2, space="PSUM"))

        c32_sb = consts.tile([128, _CONST_COLS], f32r)
        nc.sync.dma_start(out=c32_sb[:], in_=c32[:])
        ones_ap = c32_sb[0:1, _ONES_OFF : _ONES_OFF + W]

        for s in range(SPC):
            for G0, NG in CHUNKS:
                ti = inp.tile([64, 4, 2, WP], f32r, tag="ti")
                for g in range(NG):
                    G = G0 + g
                    if G == 0:
                        nc.vector.memset(ti[0:8, g, 0:1, :].bitcast(f32), 0.0)  # row -1
                        nc.sync.dma_start(
                            out=ti[0:8, g, 1:2, :],
                            in_=_dram_ap(xin, [[_HWP, IC], [1, WP]], s * _SAMP_P),
                        )
                        nc.sync.dma_start(
                            out=ti[8:64, g, :, :],
                            in_=_dram_ap(
                                xin,
                                [[2 * WP, 7], [_HWP, IC], [1, 2 * WP]],
                                s * _SAMP_P + WP,
                            ),
                        )
                    elif G == NGRP - 1:
                        nc.vector.memset(ti[32:64, g, :, :].bitcast(f32), 0.0)
                        nc.sync.dma_start(
                            out=ti[0:32, g, :, :],
                            in_=_dram_ap(
                                xin,
                                [[2 * WP, 4], [_HWP, IC], [1, 2 * WP]],
                                s * _SAMP_P + 503 * WP,
                            ),
                        )
                        nc.sync.dma_start(
                            out=ti[32:40, g, 0:1, :],
                            in_=_dram_ap(
                                xin, [[_HWP, IC], [1, WP]], s * _SAMP_P + 511 * WP
                            ),
                        )
                    else:
                        nc.sync.dma_start(
                            out=ti[:, g, :, :],
                            in_=_dram_ap(
                                xin,
                                [[2 * WP, 8], [_HWP, IC], [1, 2 * WP]],
                                s * _SAMP_P + (14 * G - 1) * WP,
                            ),
                        )

                to = outp.tile([M_PART, 4 * W], f32, tag="to")
                ps = psum.tile([M_PART, 4 * W], f32)

                for g in range(NG):
                    G = G0 + g
                    v = 1 if G == 0 else (2 if G == NGRP - 1 else 0)
                    pcol = g * W
                    nc.tensor.matmul(
                        ps[:, pcol : pcol + W],
                        c32_sb[0:1, _BIAS_OFF + s * M_PART : _BIAS_OFF + (s + 1) * M_PART],
                        ones_ap,
                        start=True,
                        stop=False,
                        skip_group_check=True,
                    )
                    for i, (dx, p) in enumerate(
                        [(dx, p) for dx in range(3) for p in range(2)]
                    ):
                        col = _a_col_pair(s, v, dx, p)
                        nc.tensor.matmul(
                            ps[:, pcol : pcol + W],
                            c32_sb[0:64, col : col + M_PART],
                            ti[:, g, p, dx : dx + W],
                            start=False,
                            stop=(i == 5),
                            skip_group_check=True,
                        )

                nc.vector.tensor_copy(out=to[:, : NG * W], in_=ps[:, : NG * W])
                for g in range(NG):
                    G = G0 + g
                    if G == NGRP - 1:
                        nc.gpsimd.dma_start(
                            out=_dram_ap(
                                yout, [[W, 8], [_HW, OC], [1, W]],
                                s * OC * _HW + 504 * W,
                            ),
                            in_=to[0:64, g * W : (g + 1) * W],
                        )
                    else:
                        nc.gpsimd.dma_start(
                            out=_dram_ap(
                                yout, [[W, TOUT], [_HW, OC], [1, W]],
                                s * OC * _HW + 14 * G * W,
                            ),
                            in_=to[:, g * W : (g + 1) * W],
                        )


def _emit_conv_f32r(nc, tc, xin, yout, c32):
    """K=128 (16 rows x 8 ic) layout, f32r taps: 3 matmuls per group,
    bias folded into the DVE PSUM->SBUF copy (tensor_scalar_add). x is
    host-padded to width 514 so every tap is N=512 (f32r needs even N)."""
    f32 = mybir.dt.float32
    f32r = mybir.dt.float32r
    import contextlib

    with contextlib.ExitStack() as ctx:
        consts = ctx.enter_context(tc.tile_pool(name="consts", bufs=1))
        inp = ctx.enter_context(tc.tile_pool(name="inp", bufs=6))
        outp = ctx.enter_context(tc.tile_pool(name="outp", bufs=4))
        psum = ctx.enter_context(tc.tile_pool(name="psum", bufs=2, space="PSUM"))

        c32_sb = consts.tile([128, _CONST_COLS], f32r)
        nc.sync.dma_start(out=c32_sb[:], in_=c32[:])
        bias_s = [
            c32_sb[0:M_PART, _BVEC_OFF + s : _BVEC_OFF + s + 1].bitcast(f32)
            for s in range(SPC)
        ]

        ci = 0  # alternate in-DMA issue between sync and scalar queues
        for s in range(SPC):
            for G0, NG in CHUNKS:
                ti = inp.tile([128, 4, WP], f32r, tag="ti")
                for g in range(NG):
                    G = G0 + g
                    eng = nc.sync if ci % 2 == 0 else nc.scalar
                    ci += 1
                    if G == 0:
                        nc.vector.memset(ti[0:32, g : g + 1, :].bitcast(f32), 0.0)
                        eng.dma_start(
                            out=ti[8:128, g : g + 1, :],
                            in_=_dram_ap(
                                xin, [[WP, 15], [_HWP, IC], [1, WP]], s * _SAMP_P
                            ),
                        )
                    elif G == NGRP - 1:
                        nc.vector.memset(ti[64:128, g : g + 1, :].bitcast(f32), 0.0)
                        eng.dma_start(
                            out=ti[0:72, g : g + 1, :],
                            in_=_dram_ap(
                                xin,
                                [[WP, 9], [_HWP, IC], [1, WP]],
                                s * _SAMP_P + 503 * WP,
                            ),
                        )
                    else:
                        eng.dma_start(
                            out=ti[:, g : g + 1, :],
                            in_=_dram_ap(
                                xin,
                                [[WP, TIN], [_HWP, IC], [1, WP]],
                                s * _SAMP_P + (14 * G - 1) * WP,
                            ),
                        )

                to = outp.tile([M_PART, 4 * W], f32, tag="to")
                ps = psum.tile([M_PART, 4 * W], f32)

                for g in range(NG):
                    G = G0 + g
                    v = 1 if G == 0 else (2 if G == NGRP - 1 else 0)
                    pcol = g * W
                    for dx in range(3):
                        col = _a_col(s, v, dx)
                        nc.tensor.matmul(
                            ps[:, pcol : pcol + W],
                            c32_sb[:, col : col + M_PART],
                            ti[:, g, dx : dx + W],
                            start=(dx == 0),
                            stop=(dx == 2),
                            skip_group_check=True,
                        )
                nc.vector.tensor_scalar_add(
                    out=to[:, : NG * W], in0=ps[:, : NG * W], scalar1=bias_s[s]
                )
                for g in range(NG):
                    G = G0 + g
                    pcol = g * W
                    if G == NGRP - 1:
                        nc.gpsimd.dma_start(
                            out=_dram_ap(
                                yout, [[W, 8], [_HW, OC], [1, W]],
                                s * OC * _HW + 504 * W,
                            ),
                            in_=to[0:64, pcol : pcol + W],
                        )
                    else:
                        nc.gpsimd.dma_start(
                            out=_dram_ap(
                                yout, [[W, TOUT], [_HW, OC], [1, W]],
                                s * OC * _HW + 14 * G * W,
                            ),
                            in_=to[:, pcol : pcol + W],
                        )


_HP = 14 * NGRP + 2  # 520 row-padded height (row -1 zero, rows 512+ zero)
_NCH = len(CHUNKS)  # 10


def _emit_conv_bf16(nc, tc, xin, yout, ca, cb):
    """Pure-bf16 conv. rel-err budget is 2e-2; bf16 rounding gives ~2e-3.

    x in DRAM is host-prepped: [SPC, HP, IC, WP] bf16, row/col
    zero-padded and (H, IC)-transposed so each 16-row group slab is one
    fully contiguous 131 KB block -> large aggregated DMA packets. Each
    group is 3 accumulating tap matmuls (K=128 = 16 rows x 8 ic,
    M=112 = 14 out rows x 8 oc, N=512); no edge variants, no bias
    matmul. Bias rides the PSUM->SBUF copy (tensor_scalar_add,
    alternating DVE/Act). Output goes out as bf16 in SBUF-tile layout
    [SPC, chunk, 112, 2048] (4 KB contiguous lines) and is unscrambled
    on host."""
    f32 = mybir.dt.float32
    bf16 = mybir.dt.bfloat16
    import contextlib

    with contextlib.ExitStack() as ctx:
        consts = ctx.enter_context(tc.tile_pool(name="consts", bufs=1))
        inp = ctx.enter_context(tc.tile_pool(name="inp", bufs=4))
        outp = ctx.enter_context(tc.tile_pool(name="outp", bufs=4))
        psum = ctx.enter_context(tc.tile_pool(name="psum", bufs=2, space="PSUM"))

        ca_sb = consts.tile([128, _CA_COLS], bf16)
        nc.sync.dma_start(out=ca_sb[:], in_=ca[:])
        cb_sb = consts.tile([M_PART, SPC], f32)
        nc.gpsimd.dma_start(out=cb_sb[:], in_=cb[:])

        ci = 0
        for s in range(SPC):
            for G0, NG in CHUNKS:
                ti = inp.tile([128, 4, WP], bf16, tag="ti")
                for g in range(NG):
                    eng = nc.sync if (ci + g) % 2 == 0 else nc.gpsimd
                    eng.dma_start(
                        out=ti[:, g : g + 1, :],
                        in_=_dram_ap(
                            xin,
                            [[WP, 128], [1, WP]],
                            (s * _HP + 14 * (G0 + g)) * IC * WP,
                        ),
                    )
                ps = psum.tile([M_PART, 4 * W], f32)
                for dx in range(3):
                    acol = (s * 3 + dx) * M_PART
                    for g in range(NG):
                        nc.tensor.matmul(
                            ps[:, g * W : (g + 1) * W],
                            ca_sb[:, acol : acol + M_PART],
                            ti[:, g, dx : dx + W],
                            start=(dx == 0),
                            stop=(dx == 2),
                            skip_group_check=True,
                        )
                to = outp.tile([M_PART, 4 * W], bf16, tag="to")
                if ci % 2 == 0:
                    nc.vector.tensor_scalar_add(
                        out=to[:, : NG * W],
                        in0=ps[:, : NG * W],
                        scalar1=cb_sb[0:M_PART, s : s + 1],
                    )
                else:
                    nc.scalar.activation(
                        out=to[:, : NG * W],
                        in_=ps[:, : NG * W],
                        func=mybir.ActivationFunctionType.Identity,
                        bias=cb_sb[0:M_PART, s : s + 1],
                    )
                oeng = nc.gpsimd if ci % 2 == 0 else nc.sync
                oeng.dma_start(
                    out=_dram_ap(
                        yout,
                        [[4 * W, M_PART], [1, NG * W]],
                        (s * _NCH + G0 // 4) * M_PART * 4 * W,
                    ),
                    in_=to[:, : NG * W],
                )
                ci += 1


def _emit_conv_s4(nc, tc, xin, yout, ca, cb):
    """4-sample-interleaved bf16 conv, one group per chunk.

    x in DRAM: [HP, IC, SPC, W] bf16 (rows zero-padded, samples
    interleaved within each (row, ic) line) -> every partition line is
    exactly 4096 B and each 16-row group slab is one 512 KB contiguous
    DMA; packets aggregate to aligned 4 KB. No width padding: the
    center tap (dx=1) runs full N=512 with start=True, the dx=0/2 taps
    run N=511 with clipped psum/rhs ranges (zero-pad semantics).
    Output: [NGRP, 112, SPC*W] bf16, unscrambled on host."""
    f32 = mybir.dt.float32
    bf16 = mybir.dt.bfloat16
    import contextlib

    SW = SPC * W  # 2048 cols per line

    with contextlib.ExitStack() as ctx:
        consts = ctx.enter_context(tc.tile_pool(name="consts", bufs=1))
        inp = ctx.enter_context(tc.tile_pool(name="inp", bufs=6))
        outp = ctx.enter_context(tc.tile_pool(name="outp", bufs=6))
        psum = ctx.enter_context(tc.tile_pool(name="psum", bufs=8, space="PSUM"))

        # slab 0 first: it gates the first matmul; consts are smaller and
        # ride the other rings concurrently.
        ti0 = inp.tile([128, SW], bf16, tag="ti")
        ca_sb = consts.tile([128, _CA_COLS], bf16)
        cb_sb = consts.tile([M_PART, SPC], f32)
        for qi, qeng in enumerate((nc.sync, nc.gpsimd, nc.scalar, nc.sync)):
            qeng.dma_start(
                out=ti0[32 * qi : 32 * qi + 32, :],
                in_=_dram_ap(xin, [[SW, 32], [1, SW]], 32 * qi * SW),
            )
        nc.scalar.dma_start(out=ca_sb[:], in_=ca[:])
        nc.gpsimd.dma_start(out=cb_sb[:], in_=cb[:])

        # PE p-state warm-up: the array needs ~3us of continuous work to
        # ramp 0.65 -> 2.4 GHz. While the first loads are in flight the
        # PE would sit idle; burn that time on junk matmuls (never-written
        # tile, no deps -> execute immediately) so the real chunk-0
        # matmuls issue at full clock.
        junk = consts.tile([128, W], bf16)
        nc.vector.memset(junk[:], 0.0)
        psw = psum.tile([M_PART, W], f32, tag="ps")
        for _ in range(14):
            nc.tensor.matmul(
                psw[:],
                junk[:, 0:M_PART],
                junk[:],
                start=True,
                stop=True,
                skip_group_check=True,
            )

        # (dx, psum col offset, rhs col offset, N)
        TAPS = [(1, 0, 0, W), (0, 1, 0, W - 1), (2, 0, 1, W - 1)]

        for G in range(NGRP):
            if G == 0:
                ti = ti0
            else:
                ti = inp.tile([128, SW], bf16, tag="ti")
                ieng = (nc.sync, nc.gpsimd) if G % 2 == 0 else (nc.gpsimd, nc.sync)
                ieng[0].dma_start(
                    out=ti[0:64, :],
                    in_=_dram_ap(xin, [[SW, 64], [1, SW]], 14 * G * IC * SW),
                )
                ieng[1].dma_start(
                    out=ti[64:128, :],
                    in_=_dram_ap(xin, [[SW, 64], [1, SW]], (14 * G * IC + 64) * SW),
                )
            for s in range(SPC):
                ps = psum.tile([M_PART, W], f32, tag="ps")
                for i, (dx, po, ro, N) in enumerate(TAPS):
                    acol = (s * 3 + dx) * M_PART
                    nc.tensor.matmul(
                        ps[:, po : po + N],
                        ca_sb[:, acol : acol + M_PART],
                        ti[:, s * W + ro : s * W + ro + N],
                        start=(i == 0),
                        stop=(i == 2),
                        skip_group_check=True,
                    )
                if s == 0:
                    to = outp.tile([M_PART, SW], bf16, tag="to")
                cp = 8 * OC if G == NGRP - 1 else M_PART
                if s % 2 == G % 2:
                    nc.vector.tensor_scalar_add(
                        out=to[0:cp, s * W : (s + 1) * W],
                        in0=ps[0:cp, :],
                        scalar1=cb_sb[0:cp, s : s + 1],
                    )
                else:
                    nc.scalar.activation(
                        out=to[0:cp, s * W : (s + 1) * W],
                        in_=ps[0:cp, :],
                        func=mybir.ActivationFunctionType.Identity,
                        bias=cb_sb[0:cp, s : s + 1],
                    )
            if G == NGRP - 1:
                # final chunk: per-sample DMAs overlap the copies, pulling
                # the last transfer off the tail
                for s in range(SPC):
                    eng = (nc.scalar, nc.sync, nc.scalar, nc.sync)[s]
                    eng.dma_start(
                        out=_dram_ap(
                            yout, [[SW, 8 * OC], [1, W]], G * M_PART * SW + s * W
                        ),
                        in_=to[0 : 8 * OC, s * W : (s + 1) * W],
                    )
            else:
                nc.scalar.dma_start(
                    out=_dram_ap(yout, [[SW, M_PART], [1, SW]], G * M_PART * SW),
                    in_=to[:],
                )


_NC_CACHE = {}


def _get_nc():
    if MODE in _NC_CACHE:
        return _NC_CACHE[MODE]
    f32 = mybir.dt.float32
    bf16 = mybir.dt.bfloat16
    nc = bass.Bass("TRN2", target_bir_lowering=False, debug=False, num_devices=NCORES)
    if MODE == "s4":
        SW = SPC * W
        xin = nc.declare_dram_parameter("x", [_HP, IC, SPC, W], bf16, isOutput=False)
        ca = nc.declare_dram_parameter("ca", [128, _CA_COLS], bf16, isOutput=False)
        cb = nc.declare_dram_parameter("cb", [M_PART, SPC], f32, isOutput=False)
        yout = nc.declare_dram_parameter(
            "y", [NGRP, M_PART, SW], bf16, isOutput=True
        )
        with TileContext(nc) as tc:
            _emit_conv_s4(nc, tc, xin, yout, ca, cb)
        _legalize_waits(nc)
        _NC_CACHE[MODE] = nc
        return nc
    if MODE == "bf16":
        xin = nc.declare_dram_parameter("x", [SPC, _HP, IC, WP], bf16, isOutput=False)
        ca = nc.declare_dram_parameter("ca", [128, _CA_COLS], bf16, isOutput=False)
        cb = nc.declare_dram_parameter("cb", [M_PART, SPC], f32, isOutput=False)
        yout = nc.declare_dram_parameter(
            "y", [SPC, _NCH, M_PART, 4 * W], bf16, isOutput=True
        )
        with TileContext(nc) as tc:
            _emit_conv_bf16(nc, tc, xin, yout, ca, cb)
        _legalize_waits(nc)
        _NC_CACHE[MODE] = nc
        return nc
    cdt = mybir.dt.float32r if MODE in ("split", "f32r", "pair") else f32
    xdt = cdt if MODE in ("f32r", "f32", "pair") else f32
    xshape = [SPC, IC, H, WP] if MODE in ("pair", "f32r") else [SPC, IC, H, W]
    xin = nc.declare_dram_parameter("x", xshape, xdt, isOutput=False)
    c32 = nc.declare_dram_parameter("c32", [128, _CONST_COLS], cdt, isOutput=False)
    cbf = None
    if MODE == "split":
        cbf = nc.declare_dram_parameter(
            "cbf", [128, 2 * _A_COLS], mybir.dt.bfloat16, isOutput=False
        )
    yout = nc.declare_dram_parameter("y", [SPC, OC, H, W], f32, isOutput=True)
    with TileContext(nc) as tc:
        if MODE == "pair":
            _emit_conv_pair(nc, tc, xin, yout, c32)
        elif MODE == "f32r":
            _emit_conv_f32r(nc, tc, xin, yout, c32)
        else:
            _emit_conv(nc, tc, xin, yout, c32, cbf)
    _legalize_waits(nc)
    _NC_CACHE[MODE] = nc
    return nc


def _build_stationaries_bf16(w):
    """w: [B, OC, IC, 3, 3] -> A: [B, 3 dx, 128, 112] bf16, no variants
    (row zero-padding lives in the DRAM x image instead)."""
    import ml_dtypes

    A = np.zeros((B, 3, 128, M_PART), np.float32)
    for dy in range(3):
        blk = w[:, :, :, dy, :].transpose(0, 3, 2, 1)  # [B, dx, IC, OC]
        for t in range(TOUT):
            A[:, :, (t + dy) * 8 : (t + dy) * 8 + 8, t * 8 : t * 8 + 8] = blk
    return A.astype(ml_dtypes.bfloat16)


def _prepare_bf16(q, x, wm1, bm1, wm2, bm2, wt, bt, wb, bb):
    import ml_dtypes

    q, x = np.asarray(q, np.float32), np.asarray(x, np.float32)
    args = [np.asarray(a, np.float32) for a in (wm1, bm1, wm2, bm2, wt, bt, wb, bb)]
    w, b = _manifold(q, *args)
    A = _build_stationaries_bf16(w)  # [B, 3, 128, 112] bf16

    xp = np.zeros((B, _HP, IC, WP), ml_dtypes.bfloat16)
    xp[:, 1 : 1 + H, :, 1 : 1 + W] = x.transpose(0, 2, 1, 3).astype(ml_dtypes.bfloat16)

    in_maps = []
    for k in range(NCORES):
        sl = slice(k * SPC, (k + 1) * SPC)
        # ca[k_part, (s*3+dx)*112 + m]
        ca = np.ascontiguousarray(
            A[sl].transpose(2, 0, 1, 3).reshape(128, _CA_COLS)
        )
        cb = np.empty((M_PART, SPC), np.float32)
        for s in range(SPC):
            cb[:, s] = np.tile(b[sl][s], TOUT)
        in_maps.append({"x": np.ascontiguousarray(xp[sl]), "ca": ca, "cb": cb})
    return _get_nc(), in_maps


def _unscramble_bf16(yd):
    """yd: [SPC, 10, 112, 2048] bf16 (one core) -> [SPC, OC, H, W] f32."""
    full = yd[:, : _NCH - 1].astype(np.float32)
    full = full.reshape(SPC, _NCH - 1, TOUT, OC, 4, W)  # [s, c, ro, oc, g, w]
    full = full.transpose(0, 3, 1, 4, 2, 5).reshape(SPC, OC, (_NCH - 1) * 4 * TOUT, W)
    last = yd[:, _NCH - 1, : 8 * OC, :W].astype(np.float32)
    last = last.reshape(SPC, 8, OC, W).transpose(0, 2, 1, 3)  # [s, oc, ro, w]
    return np.concatenate([full, last], axis=2)


def _prepare_s4(q, x, wm1, bm1, wm2, bm2, wt, bt, wb, bb):
    import ml_dtypes

    q, x = np.asarray(q, np.float32), np.asarray(x, np.float32)
    args = [np.asarray(a, np.float32) for a in (wm1, bm1, wm2, bm2, wt, bt, wb, bb)]
    w, b = _manifold(q, *args)
    A = _build_stationaries_bf16(w)  # [B, 3, 128, 112] bf16

    xb = x.astype(ml_dtypes.bfloat16)
    in_maps = []
    for k in range(NCORES):
        sl = slice(k * SPC, (k + 1) * SPC)
        ca = np.ascontiguousarray(A[sl].transpose(2, 0, 1, 3).reshape(128, _CA_COLS))
        cb = np.empty((M_PART, SPC), np.float32)
        for s in range(SPC):
            cb[:, s] = np.tile(b[sl][s], TOUT)
        # xq[row+1, ic, s, w] = x[s, ic, row, w]; rows 0 and 513.. are zero
        xq = np.zeros((_HP, IC, SPC, W), ml_dtypes.bfloat16)
        xq[1 : 1 + H] = xb[sl].transpose(2, 1, 0, 3)
        in_maps.append({"x": xq, "ca": ca, "cb": cb})
    return _get_nc(), in_maps


def _unscramble_s4(yd):
    """yd: [NGRP, 112, SPC*W] bf16 (one core) -> [SPC, OC, H, W] f32."""
    full = yd[: NGRP - 1].astype(np.float32)
    full = full.reshape(NGRP - 1, TOUT, OC, SPC, W)  # [G, ro, oc, s, w]
    full = full.transpose(3, 2, 0, 1, 4).reshape(SPC, OC, (NGRP - 1) * TOUT, W)
    last = yd[NGRP - 1, : 8 * OC].astype(np.float32)
    last = last.reshape(8, OC, SPC, W).transpose(2, 1, 0, 3)  # [s, oc, ro, w]
    return np.concatenate([full, last], axis=2)


def _prepare(q, x, wm1, bm1, wm2, bm2, wt, bt, wb, bb):
    if MODE == "s4":
        return _prepare_s4(q, x, wm1, bm1, wm2, bm2, wt, bt, wb, bb)
    if MODE == "bf16":
        return _prepare_bf16(q, x, wm1, bm1, wm2, bm2, wt, bt, wb, bb)
    q, x = np.asarray(q, np.float32), np.asarray(x, np.float32)
    args = [np.asarray(a, np.float32) for a in (wm1, bm1, wm2, bm2, wt, bt, wb, bb)]
    w, b = _manifold(q, *args)
    A = _build_stationaries_pair(w) if MODE == "pair" else _build_stationaries(w)

    if MODE in ("pair", "f32r"):
        xp = np.zeros((B, IC, H, WP), np.float32)
        xp[:, :, :, 1 : 1 + W] = x
        x = xp

    in_maps = []
    for k in range(NCORES):
        sl = slice(k * SPC, (k + 1) * SPC)
        C = _build_consts(A[sl], b[sl])
        im = {"x": np.ascontiguousarray(x[sl]), "c32": C}
        if MODE == "split":
            import ml_dtypes

            Ablock = C[:, :_A_COLS].astype(np.float32)
            Ah = Ablock.astype(ml_dtypes.bfloat16)
            Al = (Ablock - Ah.astype(np.float32)).astype(ml_dtypes.bfloat16)
            im["cbf"] = np.concatenate([Ah, Al], axis=1)
        in_maps.append(im)
    return _get_nc(), in_maps


def kernel(q, x, wm1, bm1, wm2, bm2, wt, bt, wb, bb):
    nc, in_maps = _prepare(q, x, wm1, bm1, wm2, bm2, wt, bt, wb, bb)
    br = run_bass_kernel_spmd(nc, in_maps, list(range(NCORES)))
    if MODE == "s4":
        return np.concatenate(
            [_unscramble_s4(np.asarray(r["y"])) for r in br.results], axis=0
        )
    if MODE == "bf16":
        return np.concatenate(
            [_unscramble_bf16(np.asarray(r["y"])) for r in br.results], axis=0
        )
    return np.concatenate([r["y"] for r in br.results], axis=0)

